# revision 1
# baseline (speedup 1.0000x reference)
"""CapsuleLayer dynamic-routing kernel for 8 Trainium2 NeuronCores.

Strategy: data-parallel over batch B=256 (32 per core), W replicated.
The 75 MB u_hat intermediate is never materialized:

- s-pass (per routing iteration): fold the routing coefficients c into
  W on the DVE (Wc[r,ko] = c[n,k]*W, built in 8 chunks so the PE can
  start consuming early), then s[b,ko] = x^T @ Wc as one K=9216 fp32
  matmul accumulation chain (72 K-tiles).
- squash: v = sign(s)*s^2/(1+s^2) with a fast-approx reciprocal.
- b_ij update: P[r,ko] = x^T @ v on the PE as three bf16 hi/lo chains
  (x and v split hi/lo on host/chip; the lo*lo term is dropped,
  ~1.5e-5), evacuated by a fused DVE multiply with W straight out of
  PSUM (t = W*P), then one strided DVE reduction over (i,o) yields the
  per-core partial a[n,k] in softmax-ready [n%128, (blk,k)] layout.
- cross-core: AllGather of the 46 KB partials + on-core sum (cheaper
  than AllReduce at this size), then b += a/B, softmax over k, next Wc.

Index conventions (per core):
  r = i*N + n  in [0, 9216)  — flat contraction index, 72 tiles of 128
  tile t <-> (i = t//9, blk = t%9), partition p <-> n = blk*128 + p
  ko = k*16 + o in [0, 160)

Measured on HW (8 axon-tunneled trn2 cores): relative error ~2.6e-4 vs
the fp32 jax reference; steady-state kernel time ~80-90 us (in-NEFF
repeat-delta method; single-shot adds ~15-25 us of input DMA).
"""

import numpy as np

B, N, C, O, I = 256, 1152, 10, 16, 8
NCORES = 8
BL = B // NCORES      # 32 batch per core
R = N * I             # 9216
KO = C * O            # 160
NT = R // 128         # 72 tiles
NBLK = N // 128       # 9
ITERS = 3

# PE matmul input dtype: "float32" (exact, 4 cyc/row), "bfloat16" (1 cyc/row)
MM_DTYPE = "float32"
# P-matmul in float32r (11-bit-mantissa inputs, fp32 accum, 1 cyc/row at
# N>=256). Measured on HW: saves ~10us but costs 1e-2 rel err (vs 4e-5
# fp32) through the sqrt(N)-amplified b_ij path — kept OFF.
P_F32R = False
PN = 256              # padded N for the f32r P-matmul
AG_REDUCE = True      # AllGather + on-core sum instead of AllReduce
# P-matmul as 3 bf16 hi/lo chains (xh*vh + xh*vl + xl*vh): 480 cyc/tile vs
# fp32's 640, ~1.5e-5 rel error on P (lo*lo term dropped).
P_BF16HL = True
# s-matmul with x as the MOVING operand (N=32 per tile instead of 160).
# Looks good in the cost model (no LDWEIGHTS modeling) but measured 218us
# vs 124us on HW: fp32 stationary loads of 160 cols/tile dominate. OFF.
S_FLIP = False

_BUILT = {}


def _build_program(num_devices=NCORES, collective=True, mm_dtype=None,
                   repeat=1, skip=()):
    import concourse.bass as bass
    import concourse.mybir as mybir
    import concourse.tile as tile
    import concourse.bacc as bacc

    f32 = mybir.dt.float32
    mmdt = getattr(mybir.dt, mm_dtype or MM_DTYPE)
    AX = mybir.AxisListType
    ALU = mybir.AluOpType
    ACT = mybir.ActivationFunctionType

    nc = bacc.Bacc("TRN2", target_bir_lowering=False, debug=False,
                   num_devices=num_devices)

    f32r = mybir.dt.float32r
    bf16 = mybir.dt.bfloat16
    pdt = bf16 if P_BF16HL else (f32r if P_F32R else mmdt)
    if repeat > 1:
        # distinct input signature so the PJRT/neuron compile cache can't
        # alias this build with the repeat=1 program
        nc.dram_tensor("rep_tag", [1, repeat], f32, kind="ExternalInput")
    x_s_d = nc.dram_tensor("x_s", [128, NT * BL], mmdt, kind="ExternalInput")
    x_p_d = nc.dram_tensor("x_p", [BL, R], pdt, kind="ExternalInput")
    x_pl_d = (nc.dram_tensor("x_pl", [BL, R], bf16, kind="ExternalInput")
              if P_BF16HL else None)
    w_d = nc.dram_tensor("w_s", [128, NT * KO], mmdt, kind="ExternalInput")
    v_d = nc.dram_tensor("v_out", [BL, KO], f32, kind="ExternalOutput")
    ident_d = (nc.dram_tensor("ident", [128, 128], f32, kind="ExternalInput")
               if S_FLIP else None)

    with tile.TileContext(nc) as tc:
        with (
            tc.tile_pool(name="main", bufs=1) as pool,
            tc.tile_pool(name="pp", bufs=2, space="PSUM") as pp,
            tc.tile_pool(name="ps", bufs=2, space="PSUM") as ps,
            tc.tile_pool(name="dram", bufs=2, space="DRAM") as dram,
        ):
            x_s = pool.tile([128, NT * BL], mmdt)
            x_p = pool.tile([BL, R], pdt)
            x_pl = (pool.tile([BL, R], bf16, name="x_pl")
                    if P_BF16HL else None)
            ident = (pool.tile([128, 128], f32, name="ident")
                     if S_FLIP else None)
            if S_FLIP:
                nc.sync.dma_start(ident[:, :], ident_d[:, :])
            w_sb = pool.tile([128, NT * KO], mmdt)
            # scratch reused as: a-pass W*P product, then Wc for next s-matmul
            wc = pool.tile([128, NT * KO], mmdt)
            b_ij = pool.tile([128, NBLK * C], f32)

            # chunked, interleaved loads so the iter-0 matmuls start early:
            # tile-group g (8 tiles) needs x_s cols [g*8*BL,...) and
            # w cols [g*8*KO,...)
            for g in range(NBLK):
                nc.sync.dma_start(
                    x_s[:, g * 8 * BL:(g + 1) * 8 * BL],
                    x_s_d[:, g * 8 * BL:(g + 1) * 8 * BL])
                # W in half-group chunks to spread across more DMA queues
                for h in range(2):
                    lo = g * 8 * KO + h * 4 * KO
                    nc.sync.dma_start(
                        w_sb[:, lo:lo + 4 * KO], w_d[:, lo:lo + 4 * KO])
            # x_p needed only by the a-pass (~40us in)
            for g in range(4):
                sl = slice(g * (R // 4), (g + 1) * (R // 4))
                nc.sync.dma_start(x_p[:, sl], x_p_d[:, sl])
                if P_BF16HL:
                    nc.sync.dma_start(x_pl[:, sl], x_pl_d[:, sl])

            def squash(s_psum, scale):
                """v = s*|s|/(1+s^2) = sign(s)*s^2/(1+s^2), s = scale*s_psum."""
                sq = pool.tile([BL, KO], f32, tag="sq_sq")
                sg = pool.tile([BL, KO], f32, tag="sq_sg")
                rc = pool.tile([BL, KO], f32, tag="sq_rc")
                m = pool.tile([BL, KO], f32, tag="sq_m")
                v_sb = pool.tile([BL, KO], f32, tag="sq_v")
                nc.scalar.activation(sq, s_psum, ACT.Square, scale=scale)
                nc.scalar.activation(sg, s_psum, ACT.Sign)
                d = pool.tile([BL, KO], f32, tag="sq_d")
                nc.vector.tensor_scalar_add(d, sq, 1.0)
                nc.vector.reciprocal_approx_fast(rc, d)
                nc.vector.tensor_mul(m, sq, rc)
                nc.vector.tensor_mul(v_sb, m, sg)
                if P_BF16HL:
                    vh = pool.tile([BL, KO], bf16, tag="sq_vh")
                    vl = pool.tile([BL, KO], bf16, tag="sq_vl")
                    nc.vector.tensor_copy(vh, v_sb)
                    nc.vector.tensor_sub(vl, v_sb, vh)
                    return v_sb, (vh, vl)
                if P_F32R:
                    # f32r copy of v, padded to PN cols (pads zeroed)
                    v_mm = pool.tile([BL, PN], f32r, tag="sq_vmm")
                    nc.gpsimd.memset(v_mm[:, KO:].bitcast(mybir.dt.uint32), 0)
                    nc.vector.tensor_copy(v_mm[:, :KO], v_sb)
                    return v_sb, v_mm
                if mmdt == f32:
                    return v_sb, v_sb
                v_mm = pool.tile([BL, KO], mmdt, tag="sq_vmm")
                nc.vector.tensor_copy(v_mm, v_sb)
                return v_sb, v_mm

            v_sb = None
            for _rep in range(repeat):
              for it in range(ITERS):
                # ---- s matmul: s[b, ko] = sum_r x[r, b] * Wc[r, ko] ----
                rhs = w_sb if it == 0 else wc
                scale = 0.1 if it == 0 else 1.0
                if S_FLIP:
                    # moving operand = x (N=32); out = s^T: ko 0..127 in
                    # sT[:, :BL], ko 128..159 in sT[:32, BL:2BL] (one bank)
                    # two interleaved accumulation chains in separate PSUM
                    # banks (same-bank interleaved groups are illegal)
                    sT1 = ps.tile([128, BL], f32, tag="sT1", bufs=1)
                    sT2 = ps.tile([KO - 128, BL], f32, tag="sT2", bufs=1)
                    for t in range(NT):
                        xt = x_s[:, t * BL:(t + 1) * BL]
                        nc.tensor.matmul(
                            sT1[:, :], rhs[:, t * KO:t * KO + 128], xt,
                            start=(t == 0), stop=(t == NT - 1))
                        nc.tensor.matmul(
                            sT2[:, :], rhs[:, t * KO + 128:t * KO + KO], xt,
                            start=(t == 0), stop=(t == NT - 1))
                    # evacuate (applying the iter-0 c=1/10 scale), then
                    # transpose back to [b, ko] on the PE
                    sT_sb = pool.tile([128, 2 * BL], f32, tag="sT_sb")
                    nc.scalar.activation(sT_sb[:, :BL], sT1,
                                         ACT.Copy, scale=scale)
                    nc.scalar.activation(sT_sb[:KO - 128, BL:], sT2,
                                         ACT.Copy, scale=scale)
                    stp = ps.tile([BL, KO], f32, tag="stp", bufs=1)
                    nc.tensor.transpose(stp[:, :128], sT_sb[:, :BL],
                                        ident[:, :])
                    nc.tensor.transpose(stp[:, 128:],
                                        sT_sb[:KO - 128, BL:],
                                        ident[:KO - 128, :KO - 128])
                    s_sb = pool.tile([BL, KO], f32, tag="s_sb")
                    nc.scalar.copy(s_sb, stp)
                    v_sb, v_mm = squash(s_sb, 1.0)
                else:
                    s_ps = ps.tile([BL, KO], f32, tag="s_ps")
                    if not (it > 0 and "smm12" in skip):
                        for t in range(NT):
                            nc.tensor.matmul(
                                s_ps[:, :],
                                x_s[:, t * BL:(t + 1) * BL],
                                rhs[:, t * KO:(t + 1) * KO],
                                start=(t == 0), stop=(t == NT - 1),
                            )
                    v_sb, v_mm = squash(s_ps, scale)

                if it == ITERS - 1:
                    break

                # ---- a-pass: P = x_p^T @ v per tile; t = W*P; reduce ----
                # 6 tiles per 2-bank psum chunk (3 MMs per 512-col bank so
                # each MM stays within a bank); one TT evacuates the chunk.
                if P_F32R:
                    CH, PW = 4, PN          # 4 tiles x 256 = 2 banks
                else:
                    CH, PW = 6, KO          # 2 banks of 3x160 (+32 pad)
                BPB = 512 // PW if PW <= 512 else 1   # tiles per bank
                for chunk in (range(0) if "apass" in skip
                              else range(NT // CH)):
                    p_ps = pp.tile([128, 1024], f32, tag="p_ps",
                                   bufs=3)
                    for j in range(CH):
                        t_idx = chunk * CH + j
                        off = (j // BPB) * 512 + (j % BPB) * PW
                        tsl = slice(t_idx * 128, (t_idx + 1) * 128)
                        if P_BF16HL:
                            vh, vl = v_mm
                            nc.tensor.matmul(p_ps[:, off:off + PW],
                                             x_p[:, tsl], vh[:, :],
                                             start=True, stop=False)
                            nc.tensor.matmul(p_ps[:, off:off + PW],
                                             x_p[:, tsl], vl[:, :],
                                             start=False, stop=False)
                            nc.tensor.matmul(p_ps[:, off:off + PW],
                                             x_pl[:, tsl], vh[:, :],
                                             start=False, stop=True)
                        else:
                            nc.tensor.matmul(
                                p_ps[:, off:off + PW],
                                x_p[:, tsl], v_mm[:, :],
                                start=True, stop=True,
                            )
                    sl = slice(chunk * CH * KO, (chunk + 1) * CH * KO)
                    nbank = CH // BPB
                    nc.vector.tensor_tensor(
                        out=wc[:, sl].rearrange("p (b c ko) -> p b c ko",
                                                b=nbank, c=BPB),
                        in0=p_ps.rearrange("p (b bk) -> p b bk",
                                           b=nbank)[:, :, :BPB * PW]
                            .rearrange("p b (c pw) -> p b c pw",
                                       c=BPB)[:, :, :, :KO],
                        in1=w_sb[:, sl].rearrange("p (b c ko) -> p b c ko",
                                                  b=nbank, c=BPB),
                        op=ALU.mult)

                # reduce over (i, o): wc viewed [p, blk, k, i, o].
                # DVE takes blk 0..DBLK-1; ScalarE (otherwise idle, fused
                # Copy+accum_out) takes the rest so both engines finish the
                # post-TT tail together.
                a_sb = pool.tile([128, NBLK * C], f32, tag="a_sb")
                tview = wc.rearrange("p (i blk k o) -> p blk k i o",
                                     i=I, blk=NBLK, k=C, o=O)
                av = a_sb.rearrange("p (blk k) -> p blk k", blk=NBLK)
                DBLK = 6 if "actred" in skip else NBLK
                nc.vector.reduce_sum(av[:, :DBLK, :], tview[:, :DBLK],
                                     axis=AX.XY)
                if DBLK < NBLK:
                    act_scr = pool.tile([128, I * O], f32, tag="act_scr")
                    for blk in range(DBLK, NBLK):
                        for k in range(C):
                            nc.scalar.activation(
                                act_scr.rearrange("p (i o) -> p i o", i=I),
                                tview[:, blk, k], ACT.Copy,
                                accum_out=av[:, blk, k:k + 1])

                # ---- cross-core sum of a: AllGather + local sum (AG floor
                # ~4.6us vs AllReduce ~9.7us at this size) ----
                ar_in = dram.tile([128, NBLK * C], f32, tag="ar_in")
                nc.sync.dma_start(ar_in[:, :], a_sb[:, :])
                a_red = pool.tile([128, NBLK * C], f32, tag="a_red")
                if collective and AG_REDUCE:
                    ag_out = dram.tile([num_devices * 128, NBLK * C], f32,
                                       tag="ag_out", addr_space="Shared")
                    nc.gpsimd.collective_compute(
                        "AllGather", ALU.bypass,
                        replica_groups=[list(range(num_devices))],
                        ins=[ar_in.opt()], outs=[ag_out.opt()],
                    )
                    ag_sb = pool.tile([128, num_devices * NBLK * C], f32,
                                      tag="ag_sb")
                    nc.sync.dma_start(
                        ag_sb.rearrange("p (c f) -> p c f", c=num_devices),
                        ag_out.rearrange("(c p) f -> p c f", p=128))
                    nc.vector.reduce_sum(
                        a_red,
                        ag_sb.rearrange("p (c f) -> p f c", c=num_devices),
                        axis=AX.X)
                else:
                    ar_out = dram.tile([128, NBLK * C], f32, tag="ar_out",
                                       addr_space="Shared")
                    if collective:
                        nc.gpsimd.collective_compute(
                            "AllReduce", ALU.add,
                            replica_groups=[list(range(num_devices))],
                            ins=[ar_in.opt()], outs=[ar_out.opt()],
                        )
                    else:
                        nc.sync.dma_start(ar_out[:, :], ar_in[:, :])
                    nc.sync.dma_start(a_red[:, :], ar_out[:, :])

                # ---- b_ij += a/B ; c = softmax_k(b) ----
                if it == 0:
                    nc.vector.tensor_scalar_mul(b_ij, a_red, 1.0 / B)
                else:
                    nc.vector.tensor_scalar_mul(a_red, a_red, 1.0 / B)
                    nc.vector.tensor_add(b_ij, b_ij, a_red)

                bv = b_ij.rearrange("p (blk k) -> p blk k", blk=NBLK)
                mx = pool.tile([128, NBLK], f32, tag="sm_mx")
                e_sb = pool.tile([128, NBLK * C], f32, tag="sm_e")
                sm = pool.tile([128, NBLK], f32, tag="sm_s")
                c_sb = pool.tile([128, NBLK * C], f32, tag="sm_c")
                ev = e_sb.rearrange("p (blk k) -> p blk k", blk=NBLK)
                cv = c_sb.rearrange("p (blk k) -> p blk k", blk=NBLK)
                nc.vector.reduce_max(mx, bv, axis=AX.X)
                nc.vector.tensor_sub(
                    ev, bv, mx.unsqueeze(2).broadcast_to((128, NBLK, C)))
                nc.scalar.activation(e_sb, e_sb, ACT.Exp)
                nc.vector.reduce_sum(sm, ev, axis=AX.X)
                nc.vector.reciprocal(sm, sm)
                nc.vector.tensor_mul(
                    cv, ev, sm.unsqueeze(2).broadcast_to((128, NBLK, C)))

                # ---- Wc = W * c (broadcast over o), split per i so the
                # next s-matmul can start on early chunks ----
                cb = c_sb.rearrange("p (blk k) -> p blk k", blk=NBLK) \
                    .unsqueeze(3).broadcast_to((128, NBLK, C, O))
                CW = NBLK * KO    # 1440 cols per i-chunk
                for i in range(I):
                    sl = slice(i * CW, (i + 1) * CW)
                    nc.vector.tensor_tensor(
                        out=wc[:, sl].rearrange("p (blk k o) -> p blk k o",
                                                blk=NBLK, k=C),
                        in0=w_sb[:, sl].rearrange("p (blk k o) -> p blk k o",
                                                  blk=NBLK, k=C),
                        in1=cb, op=ALU.mult)

            nc.sync.dma_start(v_d[:, :], v_sb[:, :])

    nc.compile()
    return nc


def _round_f32r(arr):
    """Round fp32 to float32r (11-bit mantissa, low 12 bits zero)."""
    u = arr.view(np.uint32)
    u = ((u.astype(np.uint64) + 0x800) & 0xFFFFF000).astype(np.uint32)
    return u.view(np.float32)


def _host_prep(x, W):
    np_mmdt = np.float32 if MM_DTYPE == "float32" else None
    import ml_dtypes
    if np_mmdt is None:
        np_mmdt = ml_dtypes.bfloat16
    W0 = np.asarray(W[0], np.float32)                   # [N, C, O, I]
    w_flat = W0.transpose(3, 0, 1, 2).reshape(R, KO)    # [(i,n), (k,o)]
    w_sb = np.ascontiguousarray(
        w_flat.reshape(NT, 128, KO).transpose(1, 0, 2).reshape(128, NT * KO)
    ).astype(np_mmdt)
    x_np = np.asarray(x, np.float32)
    x_T = x_np.transpose(2, 1, 0).reshape(R, B)         # [(i,n), b]
    in_maps = []
    for c in range(NCORES):
        xs = x_T[:, c * BL:(c + 1) * BL]
        x_s = np.ascontiguousarray(
            xs.reshape(NT, 128, BL).transpose(1, 0, 2).reshape(128, NT * BL)
        ).astype(np_mmdt)
        x_p = np.ascontiguousarray(
            x_np[c * BL:(c + 1) * BL].transpose(0, 2, 1).reshape(BL, R)
        ).astype(np_mmdt)
        if P_F32R:
            x_p = _round_f32r(x_p)
        m = {"x_s": x_s, "x_p": x_p, "w_s": w_sb}
        if S_FLIP:
            m["ident"] = np.eye(128, dtype=np.float32)
        if P_BF16HL:
            xh = x_p.astype(ml_dtypes.bfloat16)
            m["x_p"] = xh
            m["x_pl"] = (x_p - xh.astype(np.float32)).astype(ml_dtypes.bfloat16)
        in_maps.append(m)
    return in_maps


def kernel(x, W):
    from concourse import bass_utils

    if "nc" not in _BUILT:
        _BUILT["nc"] = _build_program()
    nc = _BUILT["nc"]

    in_maps = _host_prep(x, W)
    res = bass_utils.run_bass_kernel_spmd(
        nc, in_maps, core_ids=list(range(NCORES)))
    out = np.concatenate([r["v_out"] for r in res.results], axis=0)
    return out.reshape(B, C, O, 1).astype(np.float32)


if __name__ == "__main__":
    rng = np.random.default_rng(0)
    x = rng.standard_normal((B, N, I), np.float32)
    W = rng.standard_normal((1, N, C, O, I), np.float32)
    out = kernel(x, W)
    print(out.shape, out.dtype, np.abs(out).max())

